# revision 18
# baseline (speedup 1.0000x reference)
"""ConvBnA_int kernel for Trainium2 (Bass/Tile), 8 NeuronCores.

Problem: y = clip((conv3x3(x, w, pad=1) + t) >> (-n), act_min, act_max).astype(int8)
  x: (32, 128, 56, 56) f32 (integer values 0..127)
  w: (256, 128, 3, 3) f32 (integer values -128..127)
  t: (256,) f32 int-valued, n: (256,) int32 negative shifts,
  act_min/act_max: (256,) int32.

Strategy:
  - Data-parallel over batch: 4 images per core, 8 cores, no communication.
  - All values are small integers => bf16 x bf16 matmul with fp32 PSUM
    accumulation is numerically exact (products need <=16 mantissa bits,
    sums stay far below 2^24).
  - Implicit GEMM: CIN=128 is the TensorE contraction (partition) dim.
    Images are zero-padded to 58x58, flattened row-major in SBUF. Each of
    the 9 conv taps reads a 3D AP [128, 8 rows, 56 cols] slice of the
    padded image, so each PSUM tile [128 couts, 448 pix] covers exactly 8
    valid output rows (no garbage columns).
  - x ships as int8 and is cast to bf16 by a gpsimd (SWDGE) casting DMA,
    halving input DMA bytes. Image loads are chunked so the first matmuls
    start after ~1/4 of the first image has landed.
  - Per PSUM tile: 9 accumulating matmuls, then
    ACT: i32 = f32(psum) + t              (bias add + exact f32->i32)
    DVE: i32 = i32 >> shift               (per-channel arithmetic shift)
    DVE: i8  = max(min(i32, amax), amin)  (per-channel clamp + i8 cast)
"""

import numpy as np
import ml_dtypes

B, CIN, COUT, H, W, K = 32, 128, 256, 56, 56, 3
N_CORES = 8
B_LOC = B // N_CORES          # 4 images per core
PW = W + 2                    # padded width 58
PH = H + 2                    # padded height 58
NPAD = PH * PW + 2            # 3366 (+2 spare, keeps v1-compatible layout)
ROWS_PER_TILE = 8
NTILE = H // ROWS_PER_TILE    # 7 spatial tiles
TILE_N = ROWS_PER_TILE * W    # 448 valid output positions per tile
NQ = H * W                    # 3136 valid outputs per (image, channel)
CTILES = COUT // 128          # 2 cout tiles
X_CHUNKS = 4                  # image-load DMA chunks (earlier PE start)

_CACHE = {}


def _build_nc():
    import concourse.mybir as mybir
    import concourse.tile as tile
    from concourse import bacc

    dt = mybir.dt
    nc = bacc.Bacc(
        "TRN2", target_bir_lowering=False, debug=False, num_devices=N_CORES
    )

    xp = nc.dram_tensor("xp", [B_LOC, CIN, NPAD], dt.int8, kind="ExternalInput")
    wt = nc.dram_tensor("wt", [CIN, K * K * COUT], dt.bfloat16, kind="ExternalInput")
    tv = nc.dram_tensor("tv", [128, CTILES], dt.float32, kind="ExternalInput")
    sv = nc.dram_tensor("sv", [128, CTILES], dt.int32, kind="ExternalInput")
    amin = nc.dram_tensor("amin", [128, CTILES], dt.float32, kind="ExternalInput")
    amax = nc.dram_tensor("amax", [128, CTILES], dt.float32, kind="ExternalInput")
    out = nc.dram_tensor("out", [B_LOC, COUT, NQ], dt.int8, kind="ExternalOutput")

    chunk = (NPAD + X_CHUNKS - 1) // X_CHUNKS

    with tile.TileContext(nc) as tc:
        with (
            tc.tile_pool(name="const", bufs=1) as const_pool,
            tc.tile_pool(name="xin", bufs=2) as xin_pool,
            tc.tile_pool(name="psum", bufs=8, space="PSUM") as psum_pool,
            tc.tile_pool(name="ev", bufs=6) as ev_pool,
            tc.tile_pool(name="o8", bufs=6) as o8_pool,
        ):
            w_sb = const_pool.tile([CIN, K * K * COUT], dt.bfloat16)
            # chunk by tap: MM k9 only waits for its tap's slice (subtile deps);
            # alternate the two HWDGE engines (SP, ACT) for 2x queue parallelism
            for k9 in range(K * K):
                eng = nc.sync
                eng.dma_start(
                    w_sb[:, k9 * COUT : (k9 + 1) * COUT],
                    wt[:, k9 * COUT : (k9 + 1) * COUT],
                )
            tv_sb = const_pool.tile([128, CTILES], dt.float32)
            nc.sync.dma_start(tv_sb[:], tv[:, :])
            sv_sb = const_pool.tile([128, CTILES], dt.int32)
            nc.sync.dma_start(sv_sb[:], sv[:, :])
            amin_sb = const_pool.tile([128, CTILES], dt.float32)
            nc.sync.dma_start(amin_sb[:], amin[:, :])
            amax_sb = const_pool.tile([128, CTILES], dt.float32)
            nc.sync.dma_start(amax_sb[:], amax[:, :])

            for b in range(B_LOC):
                x_sb = xin_pool.tile([CIN, NPAD], dt.bfloat16)
                # first chunk small (covers st=0's rows) so PE starts early
                bounds = [0, 640, 1600, 2500, NPAD] if b == 0 else \
                         [ck * chunk for ck in range(X_CHUNKS)] + [NPAD]
                for lo, hi in zip(bounds[:-1], bounds[1:]):
                    hi = min(NPAD, hi)
                    if lo >= hi:
                        continue
                    # casting DMA (SWDGE): int8 DRAM -> bf16 SBUF
                    nc.gpsimd.dma_start(x_sb[:, lo:hi], xp[b, :, lo:hi])
                xv = x_sb[:, : PH * PW].rearrange("p (h w) -> p h w", w=PW)
                for c in range(CTILES):
                    for st in range(NTILE):
                        h0 = st * ROWS_PER_TILE
                        ps = psum_pool.tile([128, ROWS_PER_TILE, W], dt.float32)
                        for k9 in range(K * K):
                            kh, kw = divmod(k9, K)
                            nc.tensor.matmul(
                                ps[:],
                                w_sb[:, k9 * COUT + c * 128 : k9 * COUT + (c + 1) * 128],
                                xv[:, h0 + kh : h0 + kh + ROWS_PER_TILE, kw : kw + W],
                                start=(k9 == 0),
                                stop=(k9 == K * K - 1),
                            )
                        acc32 = ev_pool.tile([128, ROWS_PER_TILE, W], dt.int32)
                        nc.scalar.activation(
                            acc32[:], ps[:],
                            mybir.ActivationFunctionType.Identity,
                            bias=tv_sb[:, c : c + 1], scale=1.0,
                        )
                        sh32 = ev_pool.tile([128, ROWS_PER_TILE, W], dt.int32)
                        nc.vector.tensor_scalar(
                            sh32[:], acc32[:],
                            sv_sb[:, c : c + 1], None,
                            mybir.AluOpType.arith_shift_right,
                        )
                        # batch stores in pairs of spatial tiles: o8 spans 2
                        # tiles; store once per pair (fewer, bigger DMAs)
                        if st % 2 == 0:
                            o8 = o8_pool.tile(
                                [128, 2 * ROWS_PER_TILE, W], dt.int8, name="o8"
                            )
                        half = st % 2
                        nc.vector.tensor_scalar(
                            o8[:, half * ROWS_PER_TILE : (half + 1) * ROWS_PER_TILE],
                            sh32[:],
                            amax_sb[:, c : c + 1], amin_sb[:, c : c + 1],
                            mybir.AluOpType.min, mybir.AluOpType.max,
                        )
                        if st % 2 == 1 or st == NTILE - 1:
                            npair = 1 if st == NTILE - 1 and st % 2 == 0 else 2
                            lo = (st - npair + 1) * TILE_N
                            nc.sync.dma_start(
                                out[b, c * 128 : (c + 1) * 128,
                                    lo : lo + npair * TILE_N]
                                .rearrange("p (h w) -> p h w", w=W),
                                o8[:, : npair * ROWS_PER_TILE],
                            )
    nc.compile()
    return nc


def _prep_inputs(x, weight, t, n, act_min, act_max):
    bf16 = ml_dtypes.bfloat16
    # zero-padded 58x58 images, row-major, flattened (+2 spare elems), int8
    xp4 = np.zeros((B, CIN, PH, PW), dtype=np.int8)
    xp4[:, :, 1 : H + 1, 1 : W + 1] = x.astype(np.int8)
    xp = np.zeros((B, CIN, NPAD), dtype=np.int8)
    xp[:, :, : PH * PW] = xp4.reshape(B, CIN, PH * PW)

    # weights: [CIN, K*K, COUT] so each (tap, cout-tile) is a contiguous
    # [128, 128] stationary operand
    wt = np.ascontiguousarray(
        weight.transpose(1, 2, 3, 0).reshape(CIN, K * K * COUT)
    ).astype(bf16)

    def percore_vec(v, dtype):
        return np.ascontiguousarray(v.reshape(CTILES, 128).T).astype(dtype)

    tv = percore_vec(t, np.float32)
    sv = percore_vec(-n, np.int32)
    amin_v = percore_vec(act_min, np.float32)
    amax_v = percore_vec(act_max, np.float32)
    return xp, wt, tv, sv, amin_v, amax_v


def kernel(x, weight, t, n, act_min, act_max):
    from concourse.bass_utils import run_bass_kernel_spmd

    xp, wt, tv, sv, amin_v, amax_v = _prep_inputs(x, weight, t, n, act_min, act_max)

    if "nc" not in _CACHE:
        _CACHE["nc"] = _build_nc()
    nc = _CACHE["nc"]

    in_maps = []
    for c in range(N_CORES):
        in_maps.append(
            dict(
                xp=xp[c * B_LOC : (c + 1) * B_LOC],
                wt=wt, tv=tv, sv=sv, amin=amin_v, amax=amax_v,
            )
        )
    res = run_bass_kernel_spmd(nc, in_maps, core_ids=list(range(N_CORES)))
    outs = [r["out"] for r in res.results]
    full = np.concatenate(outs, axis=0)              # [32, 256, 3136]
    return np.ascontiguousarray(full.reshape(B, COUT, H, W))
